# revision 45
# baseline (speedup 1.0000x reference)
"""MultiHeadAttention Trainium2 kernel (8 NeuronCores).

Sharding: data-parallel over batch (2) x tensor-parallel over heads (16/4=4
head groups). Core c handles batch b = c//4 and heads 4g..4g+4 (g = c%4),
i.e. a 256-wide column slice of Wq/Wk/Wv and the matching row slice of Wo.
Each core computes a full [2048, 1024] partial output (its heads' ctx @ Wo
row-slice); the host sums the 4 partials per batch and adds the bias terms.

v3: fine-grained software pipeline. The scores+exp stream is emitted as
"beats" (4 half-array matmuls + 2 ACT exp instructions per beat) with a
filler queue of projection / ctx / out-proj matmuls popped between beats
(6 matmuls per beat) so the PE never idles: keeps the PE pstate at max and
overlaps the ACT exp time (~127us, the second-largest engine load) under
the PE's ~170us. Empirical TRN2 cadences: full-array matmul ~
free*0.42ns+35ns; 64-partition matmuls issued alternately on the two
array halves run concurrently (2x), which the scores phase exploits (dk=64).

Per-core dataflow (all fp16 operands, fp32 PSUM):
  Q.T, K.T = W.T @ xT + b (per-partition bias)     [d'=256, s] pair-packed
  V        = xT.T @ WvT (no bias; folded on host)  [s, c] + ones col/head
  scores.T = K.T_h.T @ Q.T_h (half-array pairs)    [k, q] in PSUM
  P.T      = exp(scores.T) on ACT, fp16            [k, q] SBUF
  ctx.T|r  = [V_h | 1].T @ P.T (M=65, fused rowsum), 16-step PSUM chains
  ctx_n    = ctx.T * broadcast(1/r)                [c, q] pair-packed fp16
  out_u    = ctx_n.T @ WoT                         [s, 1024] -> DRAM f32
"""

from collections import deque

import numpy as np

import concourse.bass as bass
import concourse.mybir as mybir
import concourse.tile as tile
from concourse import bacc
from concourse.bass_utils import run_bass_kernel_spmd

S = 2048          # sequence length
D = 1024          # model dim
DC = 256          # d' columns per core (4 heads x 64)
H = 4             # heads per core
DK = 64           # head dim
P = 128
F32 = mybir.dt.float32
FP16 = mybir.dt.float16
NCORES = 8

_cached = {}


def build_program():
    nc = bacc.Bacc("TRN2", target_bir_lowering=False, debug=False,
                   num_devices=NCORES)

    xqT = nc.dram_tensor("xqT", [D, S], FP16, kind="ExternalInput").ap()
    xkT = nc.dram_tensor("xkT", [D, S], FP16, kind="ExternalInput").ap()
    xvT = nc.dram_tensor("xvT", [D, S], FP16, kind="ExternalInput").ap()
    wqt = nc.dram_tensor("wqt", [D, DC], FP16, kind="ExternalInput").ap()
    wkt = nc.dram_tensor("wkt", [D, DC], FP16, kind="ExternalInput").ap()
    wvt = nc.dram_tensor("wvt", [D, DC], FP16, kind="ExternalInput").ap()
    wot = nc.dram_tensor("wot", [DC, D], FP16, kind="ExternalInput").ap()
    bqr = nc.dram_tensor("bqr", [2, P], F32, kind="ExternalInput").ap()
    bkr = nc.dram_tensor("bkr", [2, P], F32, kind="ExternalInput").ap()
    out = nc.dram_tensor("out", [S, D], F32, kind="ExternalOutput").ap()

    with tile.TileContext(nc) as tc:
        build_tile_kernel(nc, tc, xqT, xkT, xvT, wqt, wkt, wvt, wot,
                          bqr, bkr, out)

    nc.compile()
    return nc


def build_tile_kernel(nc, tc, xqT, xkT, xvT, wqt, wkt, wvt, wot,
                      bqr, bkr, out):
    from contextlib import ExitStack

    with ExitStack() as ctx:
        singles = ctx.enter_context(tc.tile_pool(name="singles", bufs=1))
        persist = ctx.enter_context(tc.tile_pool(name="persist", bufs=1))
        # PSUM: psA = scores tiles (2 banks each, 3 bufs = 6 banks for ACT
        # triple buffering); psB = everything else (1 bank each, 2 bufs).
        psA = ctx.enter_context(tc.tile_pool(name="psA", bufs=3, space="PSUM"))
        psB = ctx.enter_context(tc.tile_pool(name="psB", bufs=2, space="PSUM"))
        xT_pool = ctx.enter_context(tc.tile_pool(name="xT", bufs=3))
        pT_pool = ctx.enter_context(tc.tile_pool(name="pT", bufs=7))
        norm_pool = ctx.enter_context(tc.tile_pool(name="norm", bufs=2))
        out_sb_pool = ctx.enter_context(tc.tile_pool(name="osb", bufs=2))

        # --- weights / constants (DMA first so they are in flight early) ---
        w_k = singles.tile([P, 8, DC], FP16, tag="w_k")
        w_q = singles.tile([P, 8, DC], FP16, tag="w_q")
        w_v = singles.tile([P, 8, DC], FP16, tag="w_v")
        w_o = singles.tile([P, 2, D], FP16, tag="w_o")
        bq_t = singles.tile([P, 2], F32, tag="bq")
        bk_t = singles.tile([P, 2], F32, tag="bk")

        # --- persistent activations ---------------------------------------
        qT = persist.tile([P, 2, S], FP16, tag="qT")    # [d'%128, pair, s]
        kT = persist.tile([P, 2, S], FP16, tag="kT")
        v_sb = persist.tile([P, 16, H * (DK + 1)], FP16, tag="v_sb")
        ctxn = persist.tile([P, 2, S], FP16, tag="ctxn")  # [c%128, pair, q]

        for h in range(H):  # ones column per head for rowsum-in-matmul
            nc.vector.memset(v_sb[:, :, h * 65 + 64:h * 65 + 65], 1.0)

        # --- DMA dispatch order (critical path first: w_k + K0 halves) ----
        xchunks = {}

        def load_xT_chunk(x_dram, name, sc, split=False):
            xc = xT_pool.tile([P, 8, 512], FP16, tag="xc",
                              name=f"xc_{name}{sc}")
            ssl = slice(512 * sc, 512 * (sc + 1))
            if split:
                nc.sync.dma_start(
                    out=xc[:, 0:4, :],
                    in_=x_dram[0:512, ssl].rearrange("(t p) s -> p t s", p=P))
                nc.sync.dma_start(
                    out=xc[:, 4:8, :],
                    in_=x_dram[512:1024, ssl]
                        .rearrange("(t p) s -> p t s", p=P))
            else:
                nc.sync.dma_start(
                    out=xc, in_=x_dram[:, ssl].rearrange("(t p) s -> p t s", p=P))
            xchunks[(name, sc)] = xc

        nc.sync.dma_start(out=w_k, in_=wkt.rearrange("(t p) c -> p t c", p=P))
        load_xT_chunk(xkT, "k", 0, split=True)
        nc.sync.dma_start(out=bk_t, in_=bkr.rearrange("m p -> p m"))
        load_xT_chunk(xqT, "q", 0)
        nc.sync.dma_start(out=w_q, in_=wqt.rearrange("(t p) c -> p t c", p=P))
        nc.sync.dma_start(out=bq_t, in_=bqr.rearrange("m p -> p m"))
        nc.sync.dma_start(out=w_v, in_=wvt.rearrange("(t p) c -> p t c", p=P))
        load_xT_chunk(xkT, "k", 1)
        load_xT_chunk(xkT, "k", 2)
        load_xT_chunk(xkT, "k", 3)
        for sc in range(4):
            load_xT_chunk(xvT, "v", sc)
        for sc in range(1, 4):
            load_xT_chunk(xqT, "q", sc)
        nc.sync.dma_start(out=w_o, in_=wot.rearrange("(t p) j -> p t j", p=P))

        # --- work-unit emitters (lists of ("mm"|"aux", closure)) ----------
        uid = [0]

        def nm(pfx):
            uid[0] += 1
            return f"{pfx}_{uid[0]}"

        def emit_qk_proj(name, w_t, b_t, dest, sc):
            units = []
            xc = xchunks[(name, sc)]
            for m in range(2):
                pr = psB.tile([P, 512], F32, tag="ps1", name=nm(f"pr_{name}"))
                for dt in range(8):
                    def mm(pr=pr, m=m, dt=dt, xc=xc, w_t=w_t):
                        nc.tensor.matmul(
                            pr,
                            lhsT=w_t[:, dt, 128 * m:128 * (m + 1)],
                            rhs=xc[:, dt, :],
                            start=(dt == 0), stop=(dt == 7))
                    units.append(("mm", mm))

                def drain(pr=pr, m=m, sc=sc, dest=dest, b_t=b_t):
                    nc.vector.tensor_scalar_add(
                        dest[:, m, 512 * sc:512 * (sc + 1)], pr,
                        b_t[:, m:m + 1])
                units.append(("aux", drain))
            return units

        def emit_v_proj(sc):
            units = []
            xc = xchunks[("v", sc)]
            for st in range(4):
                pv = psB.tile([P, DC], F32, tag="ps1", name=nm("pv"))
                for dt in range(8):
                    def mm(pv=pv, st=st, dt=dt, xc=xc):
                        nc.tensor.matmul(
                            pv,
                            lhsT=xc[:, dt, 128 * st:128 * (st + 1)],
                            rhs=w_v[:, dt, :],
                            start=(dt == 0), stop=(dt == 7))
                    units.append(("mm", mm))

                def drain(pv=pv, kt=4 * sc + st):
                    nc.vector.tensor_copy(
                        v_sb[:, kt, :]
                        .rearrange("p (h c) -> p h c", h=H)[:, :, 0:DK],
                        pv.rearrange("p (h c) -> p h c", c=DK))
                units.append(("aux", drain))
            return units

        pT_tiles = {}

        def emit_scores_beat(qc, pr_i, kg):
            """One beat: 4 half-array matmuls + 2 exp ACT instructions."""
            qsl = slice(512 * qc, 512 * (qc + 1))
            h_a, h_b = 2 * pr_i, 2 * pr_i + 1
            if kg == 0:
                pT_tiles[(qc, h_a)] = pT_pool.tile(
                    [P, 16, 512], FP16, tag="pT", name=f"pT_{qc}_{h_a}")
                pT_tiles[(qc, h_b)] = pT_pool.tile(
                    [P, 16, 512], FP16, tag="pT", name=f"pT_{qc}_{h_b}")
            pT_a = pT_tiles[(qc, h_a)]
            pT_b = pT_tiles[(qc, h_b)]
            sc_a = psA.tile([P, 2, 512], F32, tag="sc", name=nm("sca"))
            sc_b = psA.tile([P, 2, 512], F32, tag="sc", name=nm("scb"))
            for khi in range(2):
                kt = 2 * kg + khi
                ksl = slice(128 * kt, 128 * (kt + 1))
                nc.tensor.matmul(sc_a[:, khi, :],
                                 lhsT=kT[0:64, pr_i, ksl],
                                 rhs=qT[0:64, pr_i, qsl])
                nc.tensor.matmul(sc_b[:, khi, :],
                                 lhsT=kT[64:128, pr_i, ksl],
                                 rhs=qT[64:128, pr_i, qsl])
            nc.scalar.activation(
                pT_a[:, 2 * kg:2 * kg + 2, :].rearrange("p a b -> p (a b)"),
                sc_a.rearrange("p a b -> p (a b)"),
                mybir.ActivationFunctionType.Exp)
            nc.scalar.activation(
                pT_b[:, 2 * kg:2 * kg + 2, :].rearrange("p a b -> p (a b)"),
                sc_b.rearrange("p a b -> p (a b)"),
                mybir.ActivationFunctionType.Exp)

        def emit_ctx(qc, h):
            """ctx chain for one head: 16 matmuls + normalize closure."""
            units = []
            qsl = slice(512 * qc, 512 * (qc + 1))
            pr_i, hp = divmod(h, 2)
            acc = psB.tile([P, 512], F32, tag="ps1", name=nm("cp"))
            for kt in range(16):
                def mm(acc=acc, qc=qc, kt=kt, h=h):
                    pT_h = pT_tiles[(qc, h)]
                    nc.tensor.matmul(
                        acc[0:65, :],
                        lhsT=v_sb[:, kt, 65 * h:65 * h + 65],
                        rhs=pT_h[:, kt, :],
                        start=(kt == 0), stop=(kt == 15))
                units.append(("mm", mm))

            def norm(acc=acc, qc=qc, h=h, hp=hp, pr_i=pr_i, qsl=qsl):
                pT_tiles.pop((qc, h))
                rs = norm_pool.tile([1, 512], F32, tag="rs", name=nm("rs"))
                nc.vector.tensor_copy(rs, acc[64:65, :])
                rc = norm_pool.tile([1, 512], F32, tag="rc", name=nm("rc"))
                nc.vector.reciprocal_approx_fast(rc, rs)
                bc = norm_pool.tile([64, 512], F32, tag="bc", name=nm("bc"))
                nc.gpsimd.partition_broadcast(bc, rc[0:1, :], channels=64)
                nc.vector.tensor_mul(
                    ctxn[64 * hp:64 * hp + 64, pr_i, qsl], acc[0:64, :], bc)
            units.append(("aux", norm))
            return units

        def emit_out(qc, st):
            """out-projection for one 128-row s-tile."""
            units = []
            ob = out_sb_pool.tile([P, D], F32, tag="ob", name=nm("ob"))
            for jc in range(2):
                op = psB.tile([P, 512], F32, tag="ps1", name=nm("op"))
                for ct in range(2):
                    def mm(op=op, st=st, jc=jc, ct=ct):
                        nc.tensor.matmul(
                            op,
                            lhsT=ctxn[:, ct, 128 * st:128 * (st + 1)],
                            rhs=w_o[:, ct, 512 * jc:512 * (jc + 1)],
                            start=(ct == 0), stop=(ct == 1))
                    units.append(("mm", mm))

                def drain(op=op, ob=ob, jc=jc, st=st):
                    nc.vector.tensor_copy(ob[:, 512 * jc:512 * (jc + 1)], op)
                    nc.sync.dma_start(
                        out=out[128 * st:128 * (st + 1),
                                512 * jc:512 * (jc + 1)],
                        in_=ob[:, 512 * jc:512 * (jc + 1)])
                units.append(("aux", drain))
            return units

        # --- filler machinery ---------------------------------------------
        fill = deque()

        def pop_fill(mm_budget):
            n = 0
            while n < mm_budget and fill:
                kind, fn = fill.popleft()
                fn()
                if kind == "mm":
                    n += 1

        # --- emission -----------------------------------------------------
        # Minimal prologue: beat (0,0,kg) only needs K chunk kg//2 + Q0, so
        # only K0+Q0 are emitted up front; K1..K3 are filled under the first
        # phase's beats at 8 pops/beat (K(sc) fully emitted before beat 2*sc
        # -- required: scores read kT via the DVE bias-add, which must come
        # earlier in the PE stream to avoid a circular queue wait).
        for _, fn in emit_qk_proj("k", w_k, bk_t, kT, 0):
            fn()
        for _, fn in emit_qk_proj("q", w_q, bq_t, qT, 0):
            fn()

        # Unlock schedule (pT bufs=6: ctx(qc,pair) completes 2 phases after
        # its scores phase; V all drained before the first ctx chain ends;
        # Q(qc) emitted fully before phase (qc,0)):
        #   (0,0): K1,K2,K3,V0   @8          (0,1): V1,V2,V3,Q1  @8
        #   (1,0): ctx(0,h0),(0,h1),Q2 @7    (1,1): ctx(0,h2),(0,h3),out(0)
        #   (2,0): ctx(1,h0),(1,h1),Q3       (2,1): ctx(1,h2),(1,h3),out(1)
        #   (3,0): ctx(2,h0),(2,h1)          (3,1): ctx(2,h2),(2,h3),out(2)
        #   tail : ctx(3,*),out(3)
        def unlock(qc, pr_i):
            if (qc, pr_i) == (0, 0):
                for sc in range(1, 4):
                    fill.extend(emit_qk_proj("k", w_k, bk_t, kT, sc))
                fill.extend(emit_v_proj(0))
                fill.extend(emit_v_proj(1))
                return
            if (qc, pr_i) == (0, 1):
                fill.extend(emit_qk_proj("q", w_q, bq_t, qT, 1))
                fill.extend(emit_v_proj(2))
                fill.extend(emit_v_proj(3))
                return
            pc = 2 * qc + pr_i - 2  # completed scores phases behind by 2
            cq, cp = divmod(pc, 2)
            fill.extend(emit_ctx(cq, 2 * cp))
            fill.extend(emit_ctx(cq, 2 * cp + 1))
            if cp == 0:
                if qc < 3 and pr_i == 0:
                    fill.extend(emit_qk_proj("q", w_q, bq_t, qT, qc + 1))
            else:
                for st in range(4 * cq, 4 * cq + 4):
                    fill.extend(emit_out(cq, st))

        budgets = {(0, 0): 12, (0, 1): 12, (1, 0): 8}
        for qc in range(4):
            for pr_i in range(2):
                unlock(qc, pr_i)
                budget = budgets.get((qc, pr_i), 7)
                for kg in range(8):
                    emit_scores_beat(qc, pr_i, kg)
                    pop_fill(budget)

        # tail
        for h in range(4):
            fill.extend(emit_ctx(3, h))
        for st in range(12, 16):
            fill.extend(emit_out(3, st))
        pop_fill(10 ** 9)


def make_in_maps(Q_input, K_input, V_input, Wq, bq, Wk, bk, Wv, Wo):
    scale = 0.125  # 1/sqrt(64), exact power of two
    xT16 = {}
    for b in range(2):
        xT16[("q", b)] = np.ascontiguousarray(Q_input[b].T).astype(np.float16)
        xT16[("k", b)] = np.ascontiguousarray(K_input[b].T).astype(np.float16)
        xT16[("v", b)] = np.ascontiguousarray(V_input[b].T).astype(np.float16)
    in_maps = []
    for c in range(NCORES):
        b, g = divmod(c, 4)
        sl = slice(DC * g, DC * (g + 1))
        in_maps.append({
            "xqT": xT16[("q", b)],
            "xkT": xT16[("k", b)],
            "xvT": xT16[("v", b)],
            "wqt": (np.ascontiguousarray(Wq[sl, :].T) * scale).astype(np.float16),
            "wkt": np.ascontiguousarray(Wk[sl, :].T).astype(np.float16),
            "wvt": np.ascontiguousarray(Wv[sl, :].T).astype(np.float16),
            "wot": np.ascontiguousarray(Wo[:, sl].T).astype(np.float16),
            "bqr": (bq[sl] * scale).reshape(2, P).astype(np.float32),
            "bkr": bk[sl].reshape(2, P).astype(np.float32),
        })
    return in_maps


def kernel(Q_input, K_input, V_input, Wq, bq, Wk, bk, Wv, bv, Wo, bo):
    if "nc" not in _cached:
        _cached["nc"] = build_program()
    nc = _cached["nc"]

    in_maps = make_in_maps(Q_input, K_input, V_input, Wq, bq, Wk, bk, Wv, Wo)
    res = run_bass_kernel_spmd(nc, in_maps, list(range(NCORES))).results
    outs = [res[c]["out"] for c in range(NCORES)]

    const = (bv.astype(np.float32) @ Wo.T.astype(np.float32)) + bo
    full = np.empty((2, S, D), np.float32)
    for b in range(2):
        acc = outs[4 * b].astype(np.float32).copy()
        for g in range(1, 4):
            acc += outs[4 * b + g]
        full[b] = acc + const
    return full


# revision 49
# speedup vs baseline: 1.0214x; 1.0214x over previous
"""MultiHeadAttention Trainium2 kernel (8 NeuronCores).

Sharding: data-parallel over batch (2) x tensor-parallel over heads (16/4=4
head groups). Core c handles batch b = c//4 and heads 4g..4g+4 (g = c%4),
i.e. a 256-wide column slice of Wq/Wk/Wv and the matching row slice of Wo.
Each core computes a full [2048, 1024] partial output (its heads' ctx @ Wo
row-slice); the host sums the 4 partials per batch and adds the bias terms.

v3: fine-grained software pipeline. The scores+exp stream is emitted as
"beats" (4 half-array matmuls + 2 ACT exp instructions per beat) with a
filler queue of projection / ctx / out-proj matmuls popped between beats
(6 matmuls per beat) so the PE never idles: keeps the PE pstate at max and
overlaps the ACT exp time (~127us, the second-largest engine load) under
the PE's ~170us. Empirical TRN2 cadences: full-array matmul ~
free*0.42ns+35ns; 64-partition matmuls issued alternately on the two
array halves run concurrently (2x), which the scores phase exploits (dk=64).

Per-core dataflow (all fp16 operands, fp32 PSUM):
  Q.T, K.T = W.T @ xT + b (per-partition bias)     [d'=256, s] pair-packed
  V        = xT.T @ WvT (no bias; folded on host)  [s, c] + ones col/head
  scores.T = K.T_h.T @ Q.T_h (half-array pairs)    [k, q] in PSUM
  P.T      = exp(scores.T) on ACT, fp16            [k, q] SBUF
  ctx.T|r  = [V_h | 1].T @ P.T (M=65, fused rowsum), 16-step PSUM chains
  ctx_n    = ctx.T * broadcast(1/r)                [c, q] pair-packed fp16
  out_u    = ctx_n.T @ WoT                         [s, 1024] -> DRAM f32
"""

from collections import deque

import numpy as np

import concourse.bass as bass
import concourse.mybir as mybir
import concourse.tile as tile
from concourse import bacc
from concourse.bass_utils import run_bass_kernel_spmd

S = 2048          # sequence length
D = 1024          # model dim
DC = 256          # d' columns per core (4 heads x 64)
H = 4             # heads per core
DK = 64           # head dim
P = 128
F32 = mybir.dt.float32
FP16 = mybir.dt.float16
NCORES = 8

_cached = {}


def build_program():
    nc = bacc.Bacc("TRN2", target_bir_lowering=False, debug=False,
                   num_devices=NCORES)

    xqT = nc.dram_tensor("xqT", [D, S], FP16, kind="ExternalInput").ap()
    xkT = nc.dram_tensor("xkT", [D, S], FP16, kind="ExternalInput").ap()
    xvT = nc.dram_tensor("xvT", [D, S], FP16, kind="ExternalInput").ap()
    wqt = nc.dram_tensor("wqt", [D, DC], FP16, kind="ExternalInput").ap()
    wkt = nc.dram_tensor("wkt", [D, DC], FP16, kind="ExternalInput").ap()
    wvt = nc.dram_tensor("wvt", [D, DC], FP16, kind="ExternalInput").ap()
    wot = nc.dram_tensor("wot", [DC, D], FP16, kind="ExternalInput").ap()
    bqr = nc.dram_tensor("bqr", [2, P], F32, kind="ExternalInput").ap()
    bkr = nc.dram_tensor("bkr", [2, P], F32, kind="ExternalInput").ap()
    out = nc.dram_tensor("out", [S, D], F32, kind="ExternalOutput").ap()

    with tile.TileContext(nc) as tc:
        build_tile_kernel(nc, tc, xqT, xkT, xvT, wqt, wkt, wvt, wot,
                          bqr, bkr, out)

    nc.compile()
    return nc


def build_tile_kernel(nc, tc, xqT, xkT, xvT, wqt, wkt, wvt, wot,
                      bqr, bkr, out):
    from contextlib import ExitStack

    with ExitStack() as ctx:
        singles = ctx.enter_context(tc.tile_pool(name="singles", bufs=1))
        persist = ctx.enter_context(tc.tile_pool(name="persist", bufs=1))
        # PSUM: psA = scores tiles (2 banks each, 3 bufs = 6 banks for ACT
        # triple buffering); psB = everything else (1 bank each, 2 bufs).
        psA = ctx.enter_context(tc.tile_pool(name="psA", bufs=3, space="PSUM"))
        psB = ctx.enter_context(tc.tile_pool(name="psB", bufs=2, space="PSUM"))
        xT_pool = ctx.enter_context(tc.tile_pool(name="xT", bufs=4))
        pT_pool = ctx.enter_context(tc.tile_pool(name="pT", bufs=6))
        norm_pool = ctx.enter_context(tc.tile_pool(name="norm", bufs=2))
        out_sb_pool = ctx.enter_context(tc.tile_pool(name="osb", bufs=2))

        # --- weights / constants (DMA first so they are in flight early) ---
        w_k = singles.tile([P, 8, DC], FP16, tag="w_k")
        w_q = singles.tile([P, 8, DC], FP16, tag="w_q")
        w_v = singles.tile([P, 8, DC], FP16, tag="w_v")
        w_o = singles.tile([P, 2, D], FP16, tag="w_o")
        bq_t = singles.tile([P, 2], F32, tag="bq")
        bk_t = singles.tile([P, 2], F32, tag="bk")

        # --- persistent activations ---------------------------------------
        qT = persist.tile([P, 2, S], FP16, tag="qT")    # [d'%128, pair, s]
        kT = persist.tile([P, 2, S], FP16, tag="kT")
        v_sb = persist.tile([P, 16, H * (DK + 1)], FP16, tag="v_sb")
        ctxn = persist.tile([P, 2, S], FP16, tag="ctxn")  # [c%128, pair, q]

        for h in range(H):  # ones column per head for rowsum-in-matmul
            nc.vector.memset(v_sb[:, :, h * 65 + 64:h * 65 + 65], 1.0)

        # --- DMA dispatch order (critical path first: w_k + K0 halves) ----
        xchunks = {}

        def load_xT_chunk(x_dram, name, sc, split=False, eng=None):
            eng = eng or nc.sync
            xc = xT_pool.tile([P, 8, 512], FP16, tag="xc",
                              name=f"xc_{name}{sc}")
            ssl = slice(512 * sc, 512 * (sc + 1))
            if split:
                eng.dma_start(
                    out=xc[:, 0:4, :],
                    in_=x_dram[0:512, ssl].rearrange("(t p) s -> p t s", p=P))
                eng.dma_start(
                    out=xc[:, 4:8, :],
                    in_=x_dram[512:1024, ssl]
                        .rearrange("(t p) s -> p t s", p=P))
            else:
                eng.dma_start(
                    out=xc, in_=x_dram[:, ssl].rearrange("(t p) s -> p t s", p=P))
            xchunks[(name, sc)] = xc

        # Head DMAs split between the two HWDGE queues (SP + ACT) to halve
        # dispatch latency. ACT-queue dispatches must be WAIT-FREE (a
        # waiting dispatch blocks the queue and stalls the upcoming exps),
        # so it only gets fresh-slot/single-tile loads.
        nc.sync.dma_start(out=w_k, in_=wkt.rearrange("(t p) c -> p t c", p=P))
        nc.scalar.dma_start(out=bk_t, in_=bkr.rearrange("m p -> p m"))
        load_xT_chunk(xkT, "k", 0, split=True)
        load_xT_chunk(xqT, "q", 0, eng=nc.scalar)
        nc.scalar.dma_start(out=w_q, in_=wqt.rearrange("(t p) c -> p t c", p=P))
        nc.scalar.dma_start(out=bq_t, in_=bqr.rearrange("m p -> p m"))
        nc.scalar.dma_start(out=w_v, in_=wvt.rearrange("(t p) c -> p t c", p=P))
        load_xT_chunk(xkT, "k", 1)
        load_xT_chunk(xkT, "k", 2, eng=nc.scalar)
        load_xT_chunk(xkT, "k", 3)
        for sc in range(4):
            load_xT_chunk(xvT, "v", sc)
        for sc in range(1, 4):
            load_xT_chunk(xqT, "q", sc)
        nc.sync.dma_start(out=w_o, in_=wot.rearrange("(t p) j -> p t j", p=P))

        # --- work-unit emitters (lists of ("mm"|"aux", closure)) ----------
        uid = [0]

        def nm(pfx):
            uid[0] += 1
            return f"{pfx}_{uid[0]}"

        def emit_qk_proj(name, w_t, b_t, dest, sc):
            units = []
            xc = xchunks[(name, sc)]
            for m in range(2):
                pr = psB.tile([P, 512], F32, tag="ps1", name=nm(f"pr_{name}"))
                for dt in range(8):
                    def mm(pr=pr, m=m, dt=dt, xc=xc, w_t=w_t):
                        nc.tensor.matmul(
                            pr,
                            lhsT=w_t[:, dt, 128 * m:128 * (m + 1)],
                            rhs=xc[:, dt, :],
                            start=(dt == 0), stop=(dt == 7))
                    units.append(("mm", mm))

                def drain(pr=pr, m=m, sc=sc, dest=dest, b_t=b_t):
                    nc.vector.tensor_scalar_add(
                        dest[:, m, 512 * sc:512 * (sc + 1)], pr,
                        b_t[:, m:m + 1])
                units.append(("aux", drain))
            return units

        def emit_v_proj(sc):
            units = []
            xc = xchunks[("v", sc)]
            for st in range(4):
                pv = psB.tile([P, DC], F32, tag="ps1", name=nm("pv"))
                for dt in range(8):
                    def mm(pv=pv, st=st, dt=dt, xc=xc):
                        nc.tensor.matmul(
                            pv,
                            lhsT=xc[:, dt, 128 * st:128 * (st + 1)],
                            rhs=w_v[:, dt, :],
                            start=(dt == 0), stop=(dt == 7))
                    units.append(("mm", mm))

                def drain(pv=pv, kt=4 * sc + st):
                    nc.vector.tensor_copy(
                        v_sb[:, kt, :]
                        .rearrange("p (h c) -> p h c", h=H)[:, :, 0:DK],
                        pv.rearrange("p (h c) -> p h c", c=DK))
                units.append(("aux", drain))
            return units

        pT_tiles = {}

        def emit_scores_beat(qc, pr_i, kg):
            """One beat: 4 half-array matmuls + 2 exp ACT instructions."""
            qsl = slice(512 * qc, 512 * (qc + 1))
            h_a, h_b = 2 * pr_i, 2 * pr_i + 1
            if kg == 0:
                pT_tiles[(qc, h_a)] = pT_pool.tile(
                    [P, 16, 512], FP16, tag="pT", name=f"pT_{qc}_{h_a}")
                pT_tiles[(qc, h_b)] = pT_pool.tile(
                    [P, 16, 512], FP16, tag="pT", name=f"pT_{qc}_{h_b}")
            pT_a = pT_tiles[(qc, h_a)]
            pT_b = pT_tiles[(qc, h_b)]
            sc_a = psA.tile([P, 2, 512], F32, tag="sc", name=nm("sca"))
            sc_b = psA.tile([P, 2, 512], F32, tag="sc", name=nm("scb"))
            for khi in range(2):
                kt = 2 * kg + khi
                ksl = slice(128 * kt, 128 * (kt + 1))
                nc.tensor.matmul(sc_a[:, khi, :],
                                 lhsT=kT[0:64, pr_i, ksl],
                                 rhs=qT[0:64, pr_i, qsl])
                nc.tensor.matmul(sc_b[:, khi, :],
                                 lhsT=kT[64:128, pr_i, ksl],
                                 rhs=qT[64:128, pr_i, qsl])
            nc.scalar.activation(
                pT_a[:, 2 * kg:2 * kg + 2, :].rearrange("p a b -> p (a b)"),
                sc_a.rearrange("p a b -> p (a b)"),
                mybir.ActivationFunctionType.Exp)
            nc.scalar.activation(
                pT_b[:, 2 * kg:2 * kg + 2, :].rearrange("p a b -> p (a b)"),
                sc_b.rearrange("p a b -> p (a b)"),
                mybir.ActivationFunctionType.Exp)

        def emit_ctx(qc, h):
            """ctx chain for one head: 16 matmuls + normalize closure."""
            units = []
            qsl = slice(512 * qc, 512 * (qc + 1))
            pr_i, hp = divmod(h, 2)
            acc = psB.tile([P, 512], F32, tag="ps1", name=nm("cp"))
            for kt in range(16):
                def mm(acc=acc, qc=qc, kt=kt, h=h):
                    pT_h = pT_tiles[(qc, h)]
                    nc.tensor.matmul(
                        acc[0:65, :],
                        lhsT=v_sb[:, kt, 65 * h:65 * h + 65],
                        rhs=pT_h[:, kt, :],
                        start=(kt == 0), stop=(kt == 15))
                units.append(("mm", mm))

            def norm(acc=acc, qc=qc, h=h, hp=hp, pr_i=pr_i, qsl=qsl):
                pT_tiles.pop((qc, h))
                rs = norm_pool.tile([1, 512], F32, tag="rs", name=nm("rs"))
                nc.vector.tensor_copy(rs, acc[64:65, :])
                rc = norm_pool.tile([1, 512], F32, tag="rc", name=nm("rc"))
                nc.vector.reciprocal_approx_fast(rc, rs)
                bc = norm_pool.tile([64, 512], F32, tag="bc", name=nm("bc"))
                nc.gpsimd.partition_broadcast(bc, rc[0:1, :], channels=64)
                nc.vector.tensor_mul(
                    ctxn[64 * hp:64 * hp + 64, pr_i, qsl], acc[0:64, :], bc)
            units.append(("aux", norm))
            return units

        def emit_out(qc, st):
            """out-projection for one 128-row s-tile."""
            units = []
            ob = out_sb_pool.tile([P, D], F32, tag="ob", name=nm("ob"))
            for jc in range(2):
                op = psB.tile([P, 512], F32, tag="ps1", name=nm("op"))
                for ct in range(2):
                    def mm(op=op, st=st, jc=jc, ct=ct):
                        nc.tensor.matmul(
                            op,
                            lhsT=ctxn[:, ct, 128 * st:128 * (st + 1)],
                            rhs=w_o[:, ct, 512 * jc:512 * (jc + 1)],
                            start=(ct == 0), stop=(ct == 1))
                    units.append(("mm", mm))

                def drain(op=op, ob=ob, jc=jc, st=st, qc=qc):
                    nc.vector.tensor_copy(ob[:, 512 * jc:512 * (jc + 1)], op)
                    # tail out-DMAs go on the (by then idle) ACT queue so
                    # the two HWDGE queues dispatch the final drains in
                    # parallel
                    eng = nc.scalar if (qc == 3 and jc == 0) else nc.sync
                    eng.dma_start(
                        out=out[128 * st:128 * (st + 1),
                                512 * jc:512 * (jc + 1)],
                        in_=ob[:, 512 * jc:512 * (jc + 1)])
                units.append(("aux", drain))
            return units

        # --- filler machinery ---------------------------------------------
        fill = deque()

        def pop_fill(mm_budget):
            n = 0
            while n < mm_budget and fill:
                kind, fn = fill.popleft()
                fn()
                if kind == "mm":
                    n += 1

        # --- emission -----------------------------------------------------
        # Minimal prologue: beat (0,0,kg) only needs K chunk kg//2 + Q0, so
        # only K0+Q0 are emitted up front; K1..K3 are filled under the first
        # phase's beats at 8 pops/beat (K(sc) fully emitted before beat 2*sc
        # -- required: scores read kT via the DVE bias-add, which must come
        # earlier in the PE stream to avoid a circular queue wait).
        for _, fn in emit_qk_proj("k", w_k, bk_t, kT, 0):
            fn()
        for _, fn in emit_qk_proj("q", w_q, bq_t, qT, 0):
            fn()

        # Unlock schedule (pT bufs=6: ctx(qc,pair) completes 2 phases after
        # its scores phase; V all drained before the first ctx chain ends;
        # Q(qc) emitted fully before phase (qc,0)):
        #   (0,0): K1,K2,K3,V0   @8          (0,1): V1,V2,V3,Q1  @8
        #   (1,0): ctx(0,h0),(0,h1),Q2 @7    (1,1): ctx(0,h2),(0,h3),out(0)
        #   (2,0): ctx(1,h0),(1,h1),Q3       (2,1): ctx(1,h2),(1,h3),out(1)
        #   (3,0): ctx(2,h0),(2,h1)          (3,1): ctx(2,h2),(2,h3),out(2)
        #   tail : ctx(3,*),out(3)
        def unlock(qc, pr_i):
            if (qc, pr_i) == (0, 0):
                for sc in range(1, 4):
                    fill.extend(emit_qk_proj("k", w_k, bk_t, kT, sc))
                fill.extend(emit_v_proj(0))
                fill.extend(emit_v_proj(1))
                return
            if (qc, pr_i) == (0, 1):
                fill.extend(emit_qk_proj("q", w_q, bq_t, qT, 1))
                fill.extend(emit_v_proj(2))
                fill.extend(emit_v_proj(3))
                return
            pc = 2 * qc + pr_i - 2  # completed scores phases behind by 2
            cq, cp = divmod(pc, 2)
            fill.extend(emit_ctx(cq, 2 * cp))
            fill.extend(emit_ctx(cq, 2 * cp + 1))
            if cp == 0:
                if qc < 3 and pr_i == 0:
                    fill.extend(emit_qk_proj("q", w_q, bq_t, qT, qc + 1))
            else:
                for st in range(4 * cq, 4 * cq + 4):
                    fill.extend(emit_out(cq, st))

        budgets = {(0, 0): 12, (0, 1): 12, (1, 0): 8}
        for qc in range(4):
            for pr_i in range(2):
                unlock(qc, pr_i)
                budget = budgets.get((qc, pr_i), 7)
                for kg in range(8):
                    emit_scores_beat(qc, pr_i, kg)
                    pop_fill(budget)

        # tail
        for h in range(4):
            fill.extend(emit_ctx(3, h))
        for st in range(12, 16):
            fill.extend(emit_out(3, st))
        pop_fill(10 ** 9)


def make_in_maps(Q_input, K_input, V_input, Wq, bq, Wk, bk, Wv, Wo):
    scale = 0.125  # 1/sqrt(64), exact power of two
    xT16 = {}
    for b in range(2):
        xT16[("q", b)] = np.ascontiguousarray(Q_input[b].T).astype(np.float16)
        xT16[("k", b)] = np.ascontiguousarray(K_input[b].T).astype(np.float16)
        xT16[("v", b)] = np.ascontiguousarray(V_input[b].T).astype(np.float16)
    in_maps = []
    for c in range(NCORES):
        b, g = divmod(c, 4)
        sl = slice(DC * g, DC * (g + 1))
        in_maps.append({
            "xqT": xT16[("q", b)],
            "xkT": xT16[("k", b)],
            "xvT": xT16[("v", b)],
            "wqt": (np.ascontiguousarray(Wq[sl, :].T) * scale).astype(np.float16),
            "wkt": np.ascontiguousarray(Wk[sl, :].T).astype(np.float16),
            "wvt": np.ascontiguousarray(Wv[sl, :].T).astype(np.float16),
            "wot": np.ascontiguousarray(Wo[:, sl].T).astype(np.float16),
            "bqr": (bq[sl] * scale).reshape(2, P).astype(np.float32),
            "bkr": bk[sl].reshape(2, P).astype(np.float32),
        })
    return in_maps


def kernel(Q_input, K_input, V_input, Wq, bq, Wk, bk, Wv, bv, Wo, bo):
    if "nc" not in _cached:
        _cached["nc"] = build_program()
    nc = _cached["nc"]

    in_maps = make_in_maps(Q_input, K_input, V_input, Wq, bq, Wk, bk, Wv, Wo)
    res = run_bass_kernel_spmd(nc, in_maps, list(range(NCORES))).results
    outs = [res[c]["out"] for c in range(NCORES)]

    const = (bv.astype(np.float32) @ Wo.T.astype(np.float32)) + bo
    full = np.empty((2, S, D), np.float32)
    for b in range(2):
        acc = outs[4 * b].astype(np.float32).copy()
        for g in range(1, 4):
            acc += outs[4 * b + g]
        full[b] = acc + const
    return full


# revision 51
# speedup vs baseline: 1.0327x; 1.0110x over previous
"""MultiHeadAttention Trainium2 kernel (8 NeuronCores).

Sharding: data-parallel over batch (2) x tensor-parallel over heads (16/4=4
head groups). Core c handles batch b = c//4 and heads 4g..4g+4 (g = c%4),
i.e. a 256-wide column slice of Wq/Wk/Wv and the matching row slice of Wo.
Each core computes a full [2048, 1024] partial output (its heads' ctx @ Wo
row-slice); the host sums the 4 partials per batch and adds the bias terms.

v3: fine-grained software pipeline. The scores+exp stream is emitted as
"beats" (4 half-array matmuls + 2 ACT exp instructions per beat) with a
filler queue of projection / ctx / out-proj matmuls popped between beats
(6 matmuls per beat) so the PE never idles: keeps the PE pstate at max and
overlaps the ACT exp time (~127us, the second-largest engine load) under
the PE's ~170us. Empirical TRN2 cadences: full-array matmul ~
free*0.42ns+35ns; 64-partition matmuls issued alternately on the two
array halves run concurrently (2x), which the scores phase exploits (dk=64).

Per-core dataflow (all fp16 operands, fp32 PSUM):
  Q.T, K.T = W.T @ xT + b (per-partition bias)     [d'=256, s] pair-packed
  V        = xT.T @ WvT (no bias; folded on host)  [s, c] + ones col/head
  scores.T = K.T_h.T @ Q.T_h (half-array pairs)    [k, q] in PSUM
  P.T      = exp(scores.T) on ACT, fp16            [k, q] SBUF
  ctx.T|r  = [V_h | 1].T @ P.T (M=65, fused rowsum), 16-step PSUM chains
  ctx_n    = ctx.T * broadcast(1/r)                [c, q] pair-packed fp16
  out_u    = ctx_n.T @ WoT                         [s, 1024] -> DRAM f32
"""

from collections import deque

import numpy as np

import concourse.bass as bass
import concourse.mybir as mybir
import concourse.tile as tile
from concourse import bacc
from concourse.bass_utils import run_bass_kernel_spmd

S = 2048          # sequence length
D = 1024          # model dim
DC = 256          # d' columns per core (4 heads x 64)
H = 4             # heads per core
DK = 64           # head dim
P = 128
F32 = mybir.dt.float32
FP16 = mybir.dt.float16
NCORES = 8

_cached = {}


def build_program():
    nc = bacc.Bacc("TRN2", target_bir_lowering=False, debug=False,
                   num_devices=NCORES)

    xqT = nc.dram_tensor("xqT", [D, S], FP16, kind="ExternalInput").ap()
    xkT = nc.dram_tensor("xkT", [D, S], FP16, kind="ExternalInput").ap()
    xvT = nc.dram_tensor("xvT", [D, S], FP16, kind="ExternalInput").ap()
    wqt = nc.dram_tensor("wqt", [D, DC], FP16, kind="ExternalInput").ap()
    wkt = nc.dram_tensor("wkt", [D, DC], FP16, kind="ExternalInput").ap()
    wvt = nc.dram_tensor("wvt", [D, DC], FP16, kind="ExternalInput").ap()
    wot = nc.dram_tensor("wot", [DC, D], FP16, kind="ExternalInput").ap()
    bqr = nc.dram_tensor("bqr", [2, P], F32, kind="ExternalInput").ap()
    bkr = nc.dram_tensor("bkr", [2, P], F32, kind="ExternalInput").ap()
    out = nc.dram_tensor("out", [S, D], F32, kind="ExternalOutput").ap()

    with tile.TileContext(nc) as tc:
        build_tile_kernel(nc, tc, xqT, xkT, xvT, wqt, wkt, wvt, wot,
                          bqr, bkr, out)

    nc.compile()
    return nc


def build_tile_kernel(nc, tc, xqT, xkT, xvT, wqt, wkt, wvt, wot,
                      bqr, bkr, out):
    from contextlib import ExitStack

    with ExitStack() as ctx:
        singles = ctx.enter_context(tc.tile_pool(name="singles", bufs=1))
        persist = ctx.enter_context(tc.tile_pool(name="persist", bufs=1))
        # PSUM: psA = scores tiles (2 banks each, 3 bufs = 6 banks for ACT
        # triple buffering); psB = everything else (1 bank each, 2 bufs).
        psA = ctx.enter_context(tc.tile_pool(name="psA", bufs=3, space="PSUM"))
        psB = ctx.enter_context(tc.tile_pool(name="psB", bufs=2, space="PSUM"))
        xT_pool = ctx.enter_context(tc.tile_pool(name="xT", bufs=4))
        pT_pool = ctx.enter_context(tc.tile_pool(name="pT", bufs=6))
        norm_pool = ctx.enter_context(tc.tile_pool(name="norm", bufs=2))
        out_sb_pool = ctx.enter_context(tc.tile_pool(name="osb", bufs=2))

        # --- weights / constants (DMA first so they are in flight early) ---
        w_k = singles.tile([P, 8, DC], FP16, tag="w_k")
        w_q = singles.tile([P, 8, DC], FP16, tag="w_q")
        w_v = singles.tile([P, 8, DC], FP16, tag="w_v")
        w_o = singles.tile([P, 2, D], FP16, tag="w_o")
        bq_t = singles.tile([P, 2], F32, tag="bq")
        bk_t = singles.tile([P, 2], F32, tag="bk")

        # --- persistent activations ---------------------------------------
        qT = persist.tile([P, 2, S], FP16, tag="qT")    # [d'%128, pair, s]
        kT = persist.tile([P, 2, S], FP16, tag="kT")
        v_sb = persist.tile([P, 16, H * (DK + 1)], FP16, tag="v_sb")
        ctxn = persist.tile([P, 2, S], FP16, tag="ctxn")  # [c%128, pair, q]

        for h in range(H):  # ones column per head for rowsum-in-matmul
            nc.vector.memset(v_sb[:, :, h * 65 + 64:h * 65 + 65], 1.0)

        # --- DMA dispatch order (critical path first: w_k + K0 halves) ----
        xchunks = {}

        def load_xT_chunk(x_dram, name, sc, split=False, eng=None):
            eng = eng or nc.sync
            xc = xT_pool.tile([P, 8, 512], FP16, tag="xc",
                              name=f"xc_{name}{sc}")
            ssl = slice(512 * sc, 512 * (sc + 1))
            if split:
                eng.dma_start(
                    out=xc[:, 0:4, :],
                    in_=x_dram[0:512, ssl].rearrange("(t p) s -> p t s", p=P))
                eng.dma_start(
                    out=xc[:, 4:8, :],
                    in_=x_dram[512:1024, ssl]
                        .rearrange("(t p) s -> p t s", p=P))
            else:
                eng.dma_start(
                    out=xc, in_=x_dram[:, ssl].rearrange("(t p) s -> p t s", p=P))
            xchunks[(name, sc)] = xc

        nc.sync.dma_start(out=w_k, in_=wkt.rearrange("(t p) c -> p t c", p=P))
        load_xT_chunk(xkT, "k", 0, split=True)
        nc.sync.dma_start(out=bk_t, in_=bkr.rearrange("m p -> p m"))
        load_xT_chunk(xqT, "q", 0)
        nc.sync.dma_start(out=w_q, in_=wqt.rearrange("(t p) c -> p t c", p=P))
        nc.sync.dma_start(out=bq_t, in_=bqr.rearrange("m p -> p m"))
        nc.sync.dma_start(out=w_v, in_=wvt.rearrange("(t p) c -> p t c", p=P))
        load_xT_chunk(xkT, "k", 1)
        load_xT_chunk(xkT, "k", 2)
        load_xT_chunk(xkT, "k", 3)
        for sc in range(4):
            load_xT_chunk(xvT, "v", sc)
        for sc in range(1, 4):
            load_xT_chunk(xqT, "q", sc)
        nc.sync.dma_start(out=w_o, in_=wot.rearrange("(t p) j -> p t j", p=P))

        # --- work-unit emitters (lists of ("mm"|"aux", closure)) ----------
        uid = [0]

        def nm(pfx):
            uid[0] += 1
            return f"{pfx}_{uid[0]}"

        def emit_qk_proj(name, w_t, b_t, dest, sc):
            units = []
            xc = xchunks[(name, sc)]
            for m in range(2):
                pr = psB.tile([P, 512], F32, tag="ps1", name=nm(f"pr_{name}"))
                for dt in range(8):
                    def mm(pr=pr, m=m, dt=dt, xc=xc, w_t=w_t):
                        nc.tensor.matmul(
                            pr,
                            lhsT=w_t[:, dt, 128 * m:128 * (m + 1)],
                            rhs=xc[:, dt, :],
                            start=(dt == 0), stop=(dt == 7))
                    units.append(("mm", mm))

                def drain(pr=pr, m=m, sc=sc, dest=dest, b_t=b_t):
                    nc.vector.tensor_scalar_add(
                        dest[:, m, 512 * sc:512 * (sc + 1)], pr,
                        b_t[:, m:m + 1])
                units.append(("aux", drain))
            return units

        def emit_v_proj(sc):
            units = []
            xc = xchunks[("v", sc)]
            for st in range(4):
                pv = psB.tile([P, DC], F32, tag="ps1", name=nm("pv"))
                for dt in range(8):
                    def mm(pv=pv, st=st, dt=dt, xc=xc):
                        nc.tensor.matmul(
                            pv,
                            lhsT=xc[:, dt, 128 * st:128 * (st + 1)],
                            rhs=w_v[:, dt, :],
                            start=(dt == 0), stop=(dt == 7))
                    units.append(("mm", mm))

                def drain(pv=pv, kt=4 * sc + st):
                    nc.vector.tensor_copy(
                        v_sb[:, kt, :]
                        .rearrange("p (h c) -> p h c", h=H)[:, :, 0:DK],
                        pv.rearrange("p (h c) -> p h c", c=DK))
                units.append(("aux", drain))
            return units

        pT_tiles = {}

        def emit_scores_beat(qc, pr_i, kg):
            """One beat: 4 half-array matmuls + 2 exp ACT instructions."""
            qsl = slice(512 * qc, 512 * (qc + 1))
            h_a, h_b = 2 * pr_i, 2 * pr_i + 1
            if kg == 0:
                pT_tiles[(qc, h_a)] = pT_pool.tile(
                    [P, 16, 512], FP16, tag="pT", name=f"pT_{qc}_{h_a}")
                pT_tiles[(qc, h_b)] = pT_pool.tile(
                    [P, 16, 512], FP16, tag="pT", name=f"pT_{qc}_{h_b}")
            pT_a = pT_tiles[(qc, h_a)]
            pT_b = pT_tiles[(qc, h_b)]
            sc_a = psA.tile([P, 2, 512], F32, tag="sc", name=nm("sca"))
            sc_b = psA.tile([P, 2, 512], F32, tag="sc", name=nm("scb"))
            for khi in range(2):
                kt = 2 * kg + khi
                ksl = slice(128 * kt, 128 * (kt + 1))
                nc.tensor.matmul(sc_a[:, khi, :],
                                 lhsT=kT[0:64, pr_i, ksl],
                                 rhs=qT[0:64, pr_i, qsl])
                nc.tensor.matmul(sc_b[:, khi, :],
                                 lhsT=kT[64:128, pr_i, ksl],
                                 rhs=qT[64:128, pr_i, qsl])
            nc.scalar.activation(
                pT_a[:, 2 * kg:2 * kg + 2, :].rearrange("p a b -> p (a b)"),
                sc_a.rearrange("p a b -> p (a b)"),
                mybir.ActivationFunctionType.Exp)
            nc.scalar.activation(
                pT_b[:, 2 * kg:2 * kg + 2, :].rearrange("p a b -> p (a b)"),
                sc_b.rearrange("p a b -> p (a b)"),
                mybir.ActivationFunctionType.Exp)

        def emit_ctx(qc, h):
            """ctx chain for one head: 16 matmuls + normalize closure."""
            units = []
            qsl = slice(512 * qc, 512 * (qc + 1))
            pr_i, hp = divmod(h, 2)
            acc = psB.tile([P, 512], F32, tag="ps1", name=nm("cp"))
            for kt in range(16):
                def mm(acc=acc, qc=qc, kt=kt, h=h):
                    pT_h = pT_tiles[(qc, h)]
                    nc.tensor.matmul(
                        acc[0:65, :],
                        lhsT=v_sb[:, kt, 65 * h:65 * h + 65],
                        rhs=pT_h[:, kt, :],
                        start=(kt == 0), stop=(kt == 15))
                units.append(("mm", mm))

            def norm(acc=acc, qc=qc, h=h, hp=hp, pr_i=pr_i, qsl=qsl):
                pT_tiles.pop((qc, h))
                rs = norm_pool.tile([1, 512], F32, tag="rs", name=nm("rs"))
                nc.vector.tensor_copy(rs, acc[64:65, :])
                rc = norm_pool.tile([1, 512], F32, tag="rc", name=nm("rc"))
                nc.vector.reciprocal_approx_fast(rc, rs)
                bc = norm_pool.tile([64, 512], F32, tag="bc", name=nm("bc"))
                nc.gpsimd.partition_broadcast(bc, rc[0:1, :], channels=64)
                nc.vector.tensor_mul(
                    ctxn[64 * hp:64 * hp + 64, pr_i, qsl], acc[0:64, :], bc)
            units.append(("aux", norm))
            return units

        def emit_out(qc, st):
            """out-projection for one 128-row s-tile."""
            units = []
            ob = out_sb_pool.tile([P, D], F32, tag="ob", name=nm("ob"))
            for jc in range(2):
                op = psB.tile([P, 512], F32, tag="ps1", name=nm("op"))
                for ct in range(2):
                    def mm(op=op, st=st, jc=jc, ct=ct):
                        nc.tensor.matmul(
                            op,
                            lhsT=ctxn[:, ct, 128 * st:128 * (st + 1)],
                            rhs=w_o[:, ct, 512 * jc:512 * (jc + 1)],
                            start=(ct == 0), stop=(ct == 1))
                    units.append(("mm", mm))

                def drain(op=op, ob=ob, jc=jc, st=st):
                    nc.vector.tensor_copy(ob[:, 512 * jc:512 * (jc + 1)], op)
                    nc.sync.dma_start(
                        out=out[128 * st:128 * (st + 1),
                                512 * jc:512 * (jc + 1)],
                        in_=ob[:, 512 * jc:512 * (jc + 1)])
                units.append(("aux", drain))
            return units

        # --- filler machinery ---------------------------------------------
        fill = deque()

        def pop_fill(mm_budget):
            n = 0
            while n < mm_budget and fill:
                kind, fn = fill.popleft()
                fn()
                if kind == "mm":
                    n += 1

        # --- emission -----------------------------------------------------
        # Minimal prologue: beat (0,0,kg) only needs K chunk kg//2 + Q0, so
        # only K0+Q0 are emitted up front; K1..K3 are filled under the first
        # phase's beats at 8 pops/beat (K(sc) fully emitted before beat 2*sc
        # -- required: scores read kT via the DVE bias-add, which must come
        # earlier in the PE stream to avoid a circular queue wait).
        for _, fn in emit_qk_proj("k", w_k, bk_t, kT, 0):
            fn()
        for _, fn in emit_qk_proj("q", w_q, bq_t, qT, 0):
            fn()

        # Unlock schedule (pT bufs=6: ctx(qc,pair) completes 2 phases after
        # its scores phase; V all drained before the first ctx chain ends;
        # Q(qc) emitted fully before phase (qc,0)):
        #   (0,0): K1,K2,K3,V0   @8          (0,1): V1,V2,V3,Q1  @8
        #   (1,0): ctx(0,h0),(0,h1),Q2 @7    (1,1): ctx(0,h2),(0,h3),out(0)
        #   (2,0): ctx(1,h0),(1,h1),Q3       (2,1): ctx(1,h2),(1,h3),out(1)
        #   (3,0): ctx(2,h0),(2,h1)          (3,1): ctx(2,h2),(2,h3),out(2)
        #   tail : ctx(3,*),out(3)
        def unlock(qc, pr_i):
            if (qc, pr_i) == (0, 0):
                for sc in range(1, 4):
                    fill.extend(emit_qk_proj("k", w_k, bk_t, kT, sc))
                fill.extend(emit_v_proj(0))
                fill.extend(emit_v_proj(1))
                return
            if (qc, pr_i) == (0, 1):
                fill.extend(emit_qk_proj("q", w_q, bq_t, qT, 1))
                fill.extend(emit_v_proj(2))
                fill.extend(emit_v_proj(3))
                return
            pc = 2 * qc + pr_i - 2  # completed scores phases behind by 2
            cq, cp = divmod(pc, 2)
            fill.extend(emit_ctx(cq, 2 * cp))
            fill.extend(emit_ctx(cq, 2 * cp + 1))
            if cp == 0:
                if qc < 3 and pr_i == 0:
                    fill.extend(emit_qk_proj("q", w_q, bq_t, qT, qc + 1))
            else:
                for st in range(4 * cq, 4 * cq + 4):
                    fill.extend(emit_out(cq, st))

        budgets = {(0, 0): 12, (0, 1): 12, (1, 0): 8}
        for qc in range(4):
            for pr_i in range(2):
                unlock(qc, pr_i)
                budget = budgets.get((qc, pr_i), 7)
                for kg in range(8):
                    emit_scores_beat(qc, pr_i, kg)
                    pop_fill(budget)

        # tail
        for h in range(4):
            fill.extend(emit_ctx(3, h))
        for st in range(12, 16):
            fill.extend(emit_out(3, st))
        pop_fill(10 ** 9)


def make_in_maps(Q_input, K_input, V_input, Wq, bq, Wk, bk, Wv, Wo):
    scale = 0.125  # 1/sqrt(64), exact power of two
    xT16 = {}
    for b in range(2):
        xT16[("q", b)] = np.ascontiguousarray(Q_input[b].T).astype(np.float16)
        xT16[("k", b)] = np.ascontiguousarray(K_input[b].T).astype(np.float16)
        xT16[("v", b)] = np.ascontiguousarray(V_input[b].T).astype(np.float16)
    in_maps = []
    for c in range(NCORES):
        b, g = divmod(c, 4)
        sl = slice(DC * g, DC * (g + 1))
        in_maps.append({
            "xqT": xT16[("q", b)],
            "xkT": xT16[("k", b)],
            "xvT": xT16[("v", b)],
            "wqt": (np.ascontiguousarray(Wq[sl, :].T) * scale).astype(np.float16),
            "wkt": np.ascontiguousarray(Wk[sl, :].T).astype(np.float16),
            "wvt": np.ascontiguousarray(Wv[sl, :].T).astype(np.float16),
            "wot": np.ascontiguousarray(Wo[:, sl].T).astype(np.float16),
            "bqr": (bq[sl] * scale).reshape(2, P).astype(np.float32),
            "bkr": bk[sl].reshape(2, P).astype(np.float32),
        })
    return in_maps


def kernel(Q_input, K_input, V_input, Wq, bq, Wk, bk, Wv, bv, Wo, bo):
    if "nc" not in _cached:
        _cached["nc"] = build_program()
    nc = _cached["nc"]

    in_maps = make_in_maps(Q_input, K_input, V_input, Wq, bq, Wk, bk, Wv, Wo)
    res = run_bass_kernel_spmd(nc, in_maps, list(range(NCORES))).results
    outs = [res[c]["out"] for c in range(NCORES)]

    const = (bv.astype(np.float32) @ Wo.T.astype(np.float32)) + bo
    full = np.empty((2, S, D), np.float32)
    for b in range(2):
        acc = outs[4 * b].astype(np.float32).copy()
        for g in range(1, 4):
            acc += outs[4 * b + g]
        full[b] = acc + const
    return full


# revision 54
# speedup vs baseline: 1.0435x; 1.0105x over previous
"""MultiHeadAttention Trainium2 kernel (8 NeuronCores).

Sharding: data-parallel over batch (2) x tensor-parallel over heads (16/4=4
head groups). Core c handles batch b = c//4 and heads 4g..4g+4 (g = c%4),
i.e. a 256-wide column slice of Wq/Wk/Wv and the matching row slice of Wo.
Each core computes a full [2048, 1024] partial output (its heads' ctx @ Wo
row-slice); the host sums the 4 partials per batch and adds the bias terms.

v4: fine-grained software pipeline. The scores+exp stream is emitted as
"beats" (4 half-array matmuls + 2 ACT exp instructions per beat) with a
filler queue of projection / ctx / out-proj matmuls popped between beats
(12/12/8/7... per beat, phase-dependent) so the PE never idles. Only
K0+Q0 are emitted before the first beat -- beat (0,0,kg) needs just K
chunk kg//2, so K1..K3/V/Q fill under the early beats and the ACT exp
stream starts at ~15us instead of ~46us. Empirical TRN2 cadences:
full-array matmul ~ free*0.42ns+35ns (LDWEIGHTS mostly shadowed in
chains); 64-partition matmuls issued alternately on the two array halves
run concurrently (~113ns effective per 512-free matmul), which scores
exploits (dk=64). Engine loads: PE ~182us busy, ACT exp ~143us
overlapped under it. Scheduling invariants (checked by static audit):
K(sc) fully emitted before beat 2*sc; Q(qc) before phase (qc,0); all V
before the first ctx matmul; ctx(qc,pair) completes within 3 phases of
its scores phase (pT bufs=6); a unit must never be popped in its own
scores phase (its matmuls would precede the exps they consume in the
in-order PE queue -> deadlock).

Per-core dataflow (all fp16 operands, fp32 PSUM):
  Q.T, K.T = W.T @ xT + b (per-partition bias)     [d'=256, s] pair-packed
  V        = xT.T @ WvT (no bias; folded on host)  [s, c] + ones col/head
  scores.T = K.T_h.T @ Q.T_h (half-array pairs)    [k, q] in PSUM
  P.T      = exp(scores.T) on ACT, fp16            [k, q] SBUF
  ctx.T|r  = [V_h | 1].T @ P.T (M=65, fused rowsum), 16-step PSUM chains
  ctx_n    = ctx.T * broadcast(1/r)                [c, q] pair-packed fp16
  out_u    = ctx_n.T @ WoT                         [s, 1024] -> DRAM f32
"""

from collections import deque

import numpy as np

import concourse.bass as bass
import concourse.mybir as mybir
import concourse.tile as tile
from concourse import bacc
from concourse.bass_utils import run_bass_kernel_spmd

S = 2048          # sequence length
D = 1024          # model dim
DC = 256          # d' columns per core (4 heads x 64)
H = 4             # heads per core
DK = 64           # head dim
P = 128
F32 = mybir.dt.float32
FP16 = mybir.dt.float16
NCORES = 8

_cached = {}


def build_program():
    nc = bacc.Bacc("TRN2", target_bir_lowering=False, debug=False,
                   num_devices=NCORES)

    xqT = nc.dram_tensor("xqT", [D, S], FP16, kind="ExternalInput").ap()
    xkT = nc.dram_tensor("xkT", [D, S], FP16, kind="ExternalInput").ap()
    xvT = nc.dram_tensor("xvT", [D, S], FP16, kind="ExternalInput").ap()
    wqt = nc.dram_tensor("wqt", [D, DC], FP16, kind="ExternalInput").ap()
    wkt = nc.dram_tensor("wkt", [D, DC], FP16, kind="ExternalInput").ap()
    wvt = nc.dram_tensor("wvt", [D, DC], FP16, kind="ExternalInput").ap()
    wot = nc.dram_tensor("wot", [DC, D], FP16, kind="ExternalInput").ap()
    bqr = nc.dram_tensor("bqr", [2, P], F32, kind="ExternalInput").ap()
    bkr = nc.dram_tensor("bkr", [2, P], F32, kind="ExternalInput").ap()
    out = nc.dram_tensor("out", [S, D], F32, kind="ExternalOutput").ap()

    with tile.TileContext(nc) as tc:
        build_tile_kernel(nc, tc, xqT, xkT, xvT, wqt, wkt, wvt, wot,
                          bqr, bkr, out)

    nc.compile()
    return nc


def build_tile_kernel(nc, tc, xqT, xkT, xvT, wqt, wkt, wvt, wot,
                      bqr, bkr, out):
    from contextlib import ExitStack

    with ExitStack() as ctx:
        singles = ctx.enter_context(tc.tile_pool(name="singles", bufs=1))
        persist = ctx.enter_context(tc.tile_pool(name="persist", bufs=1))
        # PSUM: psA = scores tiles (2 banks each, 3 bufs = 6 banks for ACT
        # triple buffering); psB = everything else (1 bank each, 2 bufs).
        psA = ctx.enter_context(tc.tile_pool(name="psA", bufs=3, space="PSUM"))
        psB = ctx.enter_context(tc.tile_pool(name="psB", bufs=2, space="PSUM"))
        xT_pool = ctx.enter_context(tc.tile_pool(name="xT", bufs=4))
        pT_pool = ctx.enter_context(tc.tile_pool(name="pT", bufs=6))
        norm_pool = ctx.enter_context(tc.tile_pool(name="norm", bufs=2))
        out_sb_pool = ctx.enter_context(tc.tile_pool(name="osb", bufs=2))

        # --- weights / constants (DMA first so they are in flight early) ---
        w_k = singles.tile([P, 8, DC], FP16, tag="w_k")
        w_q = singles.tile([P, 8, DC], FP16, tag="w_q")
        w_v = singles.tile([P, 8, DC], FP16, tag="w_v")
        w_o = singles.tile([P, 2, D], FP16, tag="w_o")
        bq_t = singles.tile([P, 2], F32, tag="bq")
        bk_t = singles.tile([P, 2], F32, tag="bk")

        # --- persistent activations ---------------------------------------
        qT = persist.tile([P, 2, S], FP16, tag="qT")    # [d'%128, pair, s]
        kT = persist.tile([P, 2, S], FP16, tag="kT")
        v_sb = persist.tile([P, 16, H * (DK + 1)], FP16, tag="v_sb")
        ctxn = persist.tile([P, 2, S], FP16, tag="ctxn")  # [c%128, pair, q]

        for h in range(H):  # ones column per head for rowsum-in-matmul
            nc.vector.memset(v_sb[:, :, h * 65 + 64:h * 65 + 65], 1.0)

        # --- DMA dispatch order (critical path first: w_k + K0 halves) ----
        xchunks = {}

        def load_xT_chunk(x_dram, name, sc, split=False, eng=None):
            eng = eng or nc.sync
            xc = xT_pool.tile([P, 8, 512], FP16, tag="xc",
                              name=f"xc_{name}{sc}")
            ssl = slice(512 * sc, 512 * (sc + 1))
            if split:
                eng.dma_start(
                    out=xc[:, 0:4, :],
                    in_=x_dram[0:512, ssl].rearrange("(t p) s -> p t s", p=P))
                eng.dma_start(
                    out=xc[:, 4:8, :],
                    in_=x_dram[512:1024, ssl]
                        .rearrange("(t p) s -> p t s", p=P))
            else:
                eng.dma_start(
                    out=xc, in_=x_dram[:, ssl].rearrange("(t p) s -> p t s", p=P))
            xchunks[(name, sc)] = xc

        nc.sync.dma_start(out=w_k, in_=wkt.rearrange("(t p) c -> p t c", p=P))
        load_xT_chunk(xkT, "k", 0, split=True)
        nc.sync.dma_start(out=bk_t, in_=bkr.rearrange("m p -> p m"))
        load_xT_chunk(xqT, "q", 0)
        nc.sync.dma_start(out=w_q, in_=wqt.rearrange("(t p) c -> p t c", p=P))
        nc.sync.dma_start(out=bq_t, in_=bqr.rearrange("m p -> p m"))
        nc.sync.dma_start(out=w_v, in_=wvt.rearrange("(t p) c -> p t c", p=P))
        load_xT_chunk(xkT, "k", 1)
        load_xT_chunk(xkT, "k", 2)
        load_xT_chunk(xkT, "k", 3)
        for sc in range(4):
            load_xT_chunk(xvT, "v", sc)
        for sc in range(1, 4):
            load_xT_chunk(xqT, "q", sc)
        nc.sync.dma_start(out=w_o, in_=wot.rearrange("(t p) j -> p t j", p=P))

        # --- work-unit emitters (lists of ("mm"|"aux", closure)) ----------
        uid = [0]

        def nm(pfx):
            uid[0] += 1
            return f"{pfx}_{uid[0]}"

        def emit_qk_proj(name, w_t, b_t, dest, sc):
            units = []
            xc = xchunks[(name, sc)]
            for m in range(2):
                pr = psB.tile([P, 512], F32, tag="ps1", name=nm(f"pr_{name}"))
                for dt in range(8):
                    def mm(pr=pr, m=m, dt=dt, xc=xc, w_t=w_t):
                        nc.tensor.matmul(
                            pr,
                            lhsT=w_t[:, dt, 128 * m:128 * (m + 1)],
                            rhs=xc[:, dt, :],
                            start=(dt == 0), stop=(dt == 7))
                    units.append(("mm", mm))

                def drain(pr=pr, m=m, sc=sc, dest=dest, b_t=b_t):
                    nc.vector.tensor_scalar_add(
                        dest[:, m, 512 * sc:512 * (sc + 1)], pr,
                        b_t[:, m:m + 1])
                units.append(("aux", drain))
            return units

        def emit_v_proj(sc):
            units = []
            xc = xchunks[("v", sc)]
            for st in range(4):
                pv = psB.tile([P, DC], F32, tag="ps1", name=nm("pv"))
                for dt in range(8):
                    def mm(pv=pv, st=st, dt=dt, xc=xc):
                        nc.tensor.matmul(
                            pv,
                            lhsT=xc[:, dt, 128 * st:128 * (st + 1)],
                            rhs=w_v[:, dt, :],
                            start=(dt == 0), stop=(dt == 7))
                    units.append(("mm", mm))

                def drain(pv=pv, kt=4 * sc + st):
                    nc.vector.tensor_copy(
                        v_sb[:, kt, :]
                        .rearrange("p (h c) -> p h c", h=H)[:, :, 0:DK],
                        pv.rearrange("p (h c) -> p h c", c=DK))
                units.append(("aux", drain))
            return units

        pT_tiles = {}

        def emit_scores_beat(qc, pr_i, kg):
            """One beat: 4 half-array matmuls + 2 exp ACT instructions."""
            qsl = slice(512 * qc, 512 * (qc + 1))
            h_a, h_b = 2 * pr_i, 2 * pr_i + 1
            if kg == 0:
                pT_tiles[(qc, h_a)] = pT_pool.tile(
                    [P, 16, 512], FP16, tag="pT", name=f"pT_{qc}_{h_a}")
                pT_tiles[(qc, h_b)] = pT_pool.tile(
                    [P, 16, 512], FP16, tag="pT", name=f"pT_{qc}_{h_b}")
            pT_a = pT_tiles[(qc, h_a)]
            pT_b = pT_tiles[(qc, h_b)]
            sc_a = psA.tile([P, 2, 512], F32, tag="sc", name=nm("sca"))
            sc_b = psA.tile([P, 2, 512], F32, tag="sc", name=nm("scb"))
            for khi in range(2):
                kt = 2 * kg + khi
                ksl = slice(128 * kt, 128 * (kt + 1))
                nc.tensor.matmul(sc_a[:, khi, :],
                                 lhsT=kT[0:64, pr_i, ksl],
                                 rhs=qT[0:64, pr_i, qsl])
                nc.tensor.matmul(sc_b[:, khi, :],
                                 lhsT=kT[64:128, pr_i, ksl],
                                 rhs=qT[64:128, pr_i, qsl])
            nc.scalar.activation(
                pT_a[:, 2 * kg:2 * kg + 2, :].rearrange("p a b -> p (a b)"),
                sc_a.rearrange("p a b -> p (a b)"),
                mybir.ActivationFunctionType.Exp)
            nc.scalar.activation(
                pT_b[:, 2 * kg:2 * kg + 2, :].rearrange("p a b -> p (a b)"),
                sc_b.rearrange("p a b -> p (a b)"),
                mybir.ActivationFunctionType.Exp)

        def emit_ctx(qc, h):
            """ctx chain for one head: 16 matmuls + normalize closure."""
            units = []
            qsl = slice(512 * qc, 512 * (qc + 1))
            pr_i, hp = divmod(h, 2)
            acc = psB.tile([P, 512], F32, tag="ps1", name=nm("cp"))
            for kt in range(16):
                def mm(acc=acc, qc=qc, kt=kt, h=h):
                    pT_h = pT_tiles[(qc, h)]
                    nc.tensor.matmul(
                        acc[0:65, :],
                        lhsT=v_sb[:, kt, 65 * h:65 * h + 65],
                        rhs=pT_h[:, kt, :],
                        start=(kt == 0), stop=(kt == 15))
                units.append(("mm", mm))

            def norm(acc=acc, qc=qc, h=h, hp=hp, pr_i=pr_i, qsl=qsl):
                pT_tiles.pop((qc, h))
                rs = norm_pool.tile([1, 512], F32, tag="rs", name=nm("rs"))
                nc.vector.tensor_copy(rs, acc[64:65, :])
                rc = norm_pool.tile([1, 512], F32, tag="rc", name=nm("rc"))
                nc.vector.reciprocal_approx_fast(rc, rs)
                bc = norm_pool.tile([64, 512], F32, tag="bc", name=nm("bc"))
                nc.gpsimd.partition_broadcast(bc, rc[0:1, :], channels=64)
                nc.vector.tensor_mul(
                    ctxn[64 * hp:64 * hp + 64, pr_i, qsl], acc[0:64, :], bc)
            units.append(("aux", norm))
            return units

        def emit_out(qc, st):
            """out-projection for one 128-row s-tile."""
            units = []
            ob = out_sb_pool.tile([P, D], F32, tag="ob", name=nm("ob"))
            for jc in range(2):
                op = psB.tile([P, 512], F32, tag="ps1", name=nm("op"))
                for ct in range(2):
                    def mm(op=op, st=st, jc=jc, ct=ct):
                        nc.tensor.matmul(
                            op,
                            lhsT=ctxn[:, ct, 128 * st:128 * (st + 1)],
                            rhs=w_o[:, ct, 512 * jc:512 * (jc + 1)],
                            start=(ct == 0), stop=(ct == 1))
                    units.append(("mm", mm))

                def drain(op=op, ob=ob, jc=jc, st=st):
                    nc.vector.tensor_copy(ob[:, 512 * jc:512 * (jc + 1)], op)
                    nc.sync.dma_start(
                        out=out[128 * st:128 * (st + 1),
                                512 * jc:512 * (jc + 1)],
                        in_=ob[:, 512 * jc:512 * (jc + 1)])
                units.append(("aux", drain))
            return units

        # --- filler machinery ---------------------------------------------
        fill = deque()

        def pop_fill(mm_budget):
            n = 0
            while n < mm_budget and fill:
                kind, fn = fill.popleft()
                fn()
                if kind == "mm":
                    n += 1

        # --- emission -----------------------------------------------------
        # Minimal prologue: beat (0,0,kg) only needs K chunk kg//2 + Q0, so
        # only K0+Q0 are emitted up front; K1..K3 are filled under the first
        # phase's beats at 8 pops/beat (K(sc) fully emitted before beat 2*sc
        # -- required: scores read kT via the DVE bias-add, which must come
        # earlier in the PE stream to avoid a circular queue wait).
        for _, fn in emit_qk_proj("k", w_k, bk_t, kT, 0):
            fn()
        for _, fn in emit_qk_proj("q", w_q, bq_t, qT, 0):
            fn()

        # Unlock schedule (pT bufs=6: ctx(qc,pair) completes 2 phases after
        # its scores phase; V all drained before the first ctx chain ends;
        # Q(qc) emitted fully before phase (qc,0)):
        #   (0,0): K1,K2,K3,V0   @8          (0,1): V1,V2,V3,Q1  @8
        #   (1,0): ctx(0,h0),(0,h1),Q2 @7    (1,1): ctx(0,h2),(0,h3),out(0)
        #   (2,0): ctx(1,h0),(1,h1),Q3       (2,1): ctx(1,h2),(1,h3),out(1)
        #   (3,0): ctx(2,h0),(2,h1)          (3,1): ctx(2,h2),(2,h3),out(2)
        #   tail : ctx(3,*),out(3)
        def unlock(qc, pr_i):
            if (qc, pr_i) == (0, 0):
                for sc in range(1, 4):
                    fill.extend(emit_qk_proj("k", w_k, bk_t, kT, sc))
                fill.extend(emit_v_proj(0))
                fill.extend(emit_v_proj(1))
                return
            if (qc, pr_i) == (0, 1):
                fill.extend(emit_qk_proj("q", w_q, bq_t, qT, 1))
                fill.extend(emit_v_proj(2))
                fill.extend(emit_v_proj(3))
                return
            if (qc, pr_i) == (3, 0):
                # late compression: qc2 pair-1 ctx pops +1 phase after its
                # scores phase (safe: its exps are 1+ phases old by pop
                # time), filling what would otherwise be a 24-slot hole
                fill.extend(emit_ctx(2, 0))
                fill.extend(emit_ctx(2, 1))
                fill.extend(emit_ctx(2, 2))
                fill.extend(emit_ctx(2, 3))
                return
            if (qc, pr_i) == (3, 1):
                for st in range(8, 12):
                    fill.extend(emit_out(2, st))
                fill.extend(emit_ctx(3, 0))
                fill.extend(emit_ctx(3, 1))
                return
            pc = 2 * qc + pr_i - 2  # completed scores phases behind by 2
            cq, cp = divmod(pc, 2)
            fill.extend(emit_ctx(cq, 2 * cp))
            fill.extend(emit_ctx(cq, 2 * cp + 1))
            if cp == 0:
                if qc < 3 and pr_i == 0:
                    fill.extend(emit_qk_proj("q", w_q, bq_t, qT, qc + 1))
            else:
                for st in range(4 * cq, 4 * cq + 4):
                    fill.extend(emit_out(cq, st))

        budgets = {(0, 0): 12, (0, 1): 12, (1, 0): 8, (3, 0): 8}
        for qc in range(4):
            for pr_i in range(2):
                unlock(qc, pr_i)
                budget = budgets.get((qc, pr_i), 7)
                for kg in range(8):
                    emit_scores_beat(qc, pr_i, kg)
                    pop_fill(budget)

        # tail
        fill.extend(emit_ctx(3, 2))
        fill.extend(emit_ctx(3, 3))
        for st in range(12, 16):
            fill.extend(emit_out(3, st))
        pop_fill(10 ** 9)


def make_in_maps(Q_input, K_input, V_input, Wq, bq, Wk, bk, Wv, Wo):
    scale = 0.125  # 1/sqrt(64), exact power of two
    xT16 = {}
    for b in range(2):
        xT16[("q", b)] = np.ascontiguousarray(Q_input[b].T).astype(np.float16)
        xT16[("k", b)] = np.ascontiguousarray(K_input[b].T).astype(np.float16)
        xT16[("v", b)] = np.ascontiguousarray(V_input[b].T).astype(np.float16)
    in_maps = []
    for c in range(NCORES):
        b, g = divmod(c, 4)
        sl = slice(DC * g, DC * (g + 1))
        in_maps.append({
            "xqT": xT16[("q", b)],
            "xkT": xT16[("k", b)],
            "xvT": xT16[("v", b)],
            "wqt": (np.ascontiguousarray(Wq[sl, :].T) * scale).astype(np.float16),
            "wkt": np.ascontiguousarray(Wk[sl, :].T).astype(np.float16),
            "wvt": np.ascontiguousarray(Wv[sl, :].T).astype(np.float16),
            "wot": np.ascontiguousarray(Wo[:, sl].T).astype(np.float16),
            "bqr": (bq[sl] * scale).reshape(2, P).astype(np.float32),
            "bkr": bk[sl].reshape(2, P).astype(np.float32),
        })
    return in_maps


def kernel(Q_input, K_input, V_input, Wq, bq, Wk, bk, Wv, bv, Wo, bo):
    if "nc" not in _cached:
        _cached["nc"] = build_program()
    nc = _cached["nc"]

    in_maps = make_in_maps(Q_input, K_input, V_input, Wq, bq, Wk, bk, Wv, Wo)
    res = run_bass_kernel_spmd(nc, in_maps, list(range(NCORES))).results
    outs = [res[c]["out"] for c in range(NCORES)]

    const = (bv.astype(np.float32) @ Wo.T.astype(np.float32)) + bo
    full = np.empty((2, S, D), np.float32)
    for b in range(2):
        acc = outs[4 * b].astype(np.float32).copy()
        for g in range(1, 4):
            acc += outs[4 * b + g]
        full[b] = acc + const
    return full
